# revision 19
# baseline (speedup 1.0000x reference)
"""Trainium2 Bass kernel for a GPT-style decoder block (d=768, H=12, S=4096, FFN=3072).

Sharding: 8-way SPMD over query-row blocks, cyclic assignment — core c owns the
four 128-row blocks {c, c+8, c+16, c+24}.  Every core computes LN1 + K/V
projections for the full sequence locally (no collectives), then causal
attention + output projection + LN2 + FFN for its own 512 query rows.

All per-core variation is carried by input *data* (own-rows tensor, mask-bias
table); the program itself is identical on all 8 cores and fully static.

Matmuls run with bf16 operands and fp32 PSUM accumulation (measured pipeline
error vs the fp32 reference: ~1e-3 absmax-relative).  LayerNorm, softmax,
GELU and residuals are fp32.
"""

import os
import sys
from contextlib import ExitStack

import numpy as np
import ml_dtypes

for _p in ("/opt/trn_rl_repo", "/opt/pypackages"):
    if os.path.isdir(_p) and _p not in sys.path:
        sys.path.append(_p)

import concourse.bacc as bacc
import concourse.tile as tile
from concourse import mybir
from concourse.bass_utils import run_bass_kernel_spmd
from concourse.masks import make_identity

P = 128
D = 768
DT = D // P            # 6 d-tiles
H = 12
HD = 64
S = 4096
NB = S // P            # 32 sequence blocks
HID = 3072
FT = HID // P          # 24 hidden f-tiles
QB = 4                 # q-blocks per core
NCORES = 8
EPS = 1e-5
NEG = -1.0e9
SCALE = 0.125          # 1/sqrt(HD)
TOFF = [0, 8, 24, 48]  # flat (i, j) pair index offsets; 80 static pairs total
NPAIR = 80
HD1 = HD + 1           # V columns + appended ones column (softmax denominator)

F32 = mybir.dt.float32
BF16 = mybir.dt.bfloat16
AF = mybir.ActivationFunctionType

_CACHE = {}

# CoreSim does not implement the Gelu activation table; this flag substitutes
# x*sigmoid(1.702x) so the simulator can validate kernel structure.  Hardware
# runs always use the real (erf) Gelu.
GELU_SIGMOID_APPROX = False


def _build(trivial_affine=False):
    nc = bacc.Bacc("TRN2", target_bir_lowering=False, debug=False,
                   num_devices=NCORES)

    # ---- DRAM I/O ----
    x_d = nc.dram_tensor("x_full", [S, D], BF16, kind="ExternalInput").ap()
    xq_d = nc.dram_tensor("xq", [QB * P, D], F32, kind="ExternalInput").ap()
    wq_d = nc.dram_tensor("wq", [D, D], BF16, kind="ExternalInput").ap()
    wk_d = nc.dram_tensor("wk", [D, D], BF16, kind="ExternalInput").ap()
    wv_d = nc.dram_tensor("wv", [D, D], BF16, kind="ExternalInput").ap()
    wo_d = nc.dram_tensor("wo", [D, D], BF16, kind="ExternalInput").ap()
    w1_d = nc.dram_tensor("w1", [D, HID], BF16, kind="ExternalInput").ap()
    w2_d = nc.dram_tensor("w2", [HID, D], BF16, kind="ExternalInput").ap()
    g1_d = nc.dram_tensor("g1", [1, D], F32, kind="ExternalInput").ap()
    be1_d = nc.dram_tensor("be1", [1, D], F32, kind="ExternalInput").ap()
    g2_d = nc.dram_tensor("g2", [1, D], F32, kind="ExternalInput").ap()
    be2_d = nc.dram_tensor("be2", [1, D], F32, kind="ExternalInput").ap()
    bq_d = nc.dram_tensor("bq_t", [P, DT], F32, kind="ExternalInput").ap()
    bk_d = nc.dram_tensor("bk_t", [P, DT], F32, kind="ExternalInput").ap()
    bv_d = nc.dram_tensor("bv_r", [1, D], F32, kind="ExternalInput").ap()
    bo_d = nc.dram_tensor("bo_t", [P, DT], F32, kind="ExternalInput").ap()
    b1_d = nc.dram_tensor("b1_t", [P, FT], F32, kind="ExternalInput").ap()
    b2_d = nc.dram_tensor("b2_t", [P, DT], F32, kind="ExternalInput").ap()
    selb_d = nc.dram_tensor("selbias", [P, NPAIR], F32, kind="ExternalInput").ap()
    killm_d = nc.dram_tensor("killm", [P, NPAIR * P], BF16,
                             kind="ExternalInput").ap()
    tri_d = nc.dram_tensor("tri", [P, P], BF16, kind="ExternalInput").ap()
    out_d = nc.dram_tensor("out", [QB * P, D], F32, kind="ExternalOutput").ap()

    with tile.TileContext(nc) as tc, ExitStack() as root:
        singles = root.enter_context(tc.tile_pool(name="singles", bufs=1))

        # ---- constants + activations resident for the whole kernel ----
        ident = singles.tile([P, P], F32)
        make_identity(nc, ident)
        ident_bf = singles.tile([P, P], BF16)
        make_identity(nc, ident_bf)
        eps_t = singles.tile([P, 1], F32)
        nc.vector.memset(eps_t, EPS)
        bq_t = singles.tile([P, DT], F32)
        nc.sync.dma_start(out=bq_t, in_=bq_d)
        bk_t = singles.tile([P, DT], F32)
        nc.sync.dma_start(out=bk_t, in_=bk_d)
        bo_t = singles.tile([P, DT], F32)
        nc.sync.dma_start(out=bo_t, in_=bo_d)
        b1_t = singles.tile([P, FT], F32)
        nc.sync.dma_start(out=b1_t, in_=b1_d)
        b2_t = singles.tile([P, DT], F32)
        nc.sync.dma_start(out=b2_t, in_=b2_d)
        tri_t = singles.tile([P, P], BF16)
        nc.sync.dma_start(out=tri_t, in_=tri_d)

        xq_sb = singles.tile([P, QB, D], F32)        # raw own rows (residual)
        nc.sync.dma_start(out=xq_sb, in_=xq_d.rearrange("(r p) f -> p r f", p=P))
        lnq_T = singles.tile([P, DT, QB * P], BF16)  # LN1(own rows)^T
        q_T = singles.tile([P, DT, QB * P], BF16)    # Q^T (feature-major)
        attn_sb = singles.tile([P, QB, D], BF16)     # attention output

        def layer_norm(pool, x_ap, g_tile, b_tile, out_ap):
            stats = pool.tile([P, 3, nc.vector.BN_STATS_DIM], F32, tag="lns")
            mv = pool.tile([P, nc.vector.BN_AGGR_DIM], F32, tag="lnm")
            xg = x_ap.rearrange("p (g f) -> p g f", g=3)
            for g in range(3):
                nc.vector.bn_stats(out=stats[:, g, :], in_=xg[:, g, :])
            nc.vector.bn_aggr(out=mv, in_=stats)
            rstd = pool.tile([P, 1], F32, tag="lnr")
            nc.scalar.activation(out=rstd, in_=mv[:, 1:2], func=AF.Sqrt,
                                 bias=eps_t, scale=1.0)
            nc.vector.reciprocal(out=rstd, in_=rstd)
            nc.vector.tensor_scalar(out=out_ap, in0=x_ap, scalar1=mv[:, 0:1],
                                    scalar2=rstd, op0=mybir.AluOpType.subtract,
                                    op1=mybir.AluOpType.mult)
            if not trivial_affine:
                nc.vector.tensor_mul(out=out_ap, in0=out_ap, in1=g_tile)
                nc.vector.tensor_add(out=out_ap, in0=out_ap, in1=b_tile)

        def transpose_block(ptr_pool, dst_ap, src_ap):
            pt = ptr_pool.tile([P, P], F32, tag="tr")
            nc.tensor.transpose(pt, src_ap, ident)
            nc.scalar.activation(out=dst_ap, in_=pt, func=AF.Copy)

        def transpose_row(ptr_pool, dst_ap, src_ap):
            # Transpose a [128, 768] row-major block into dst [128, 6, 128]
            # (feature-major) with one batched PSUM drain.  bf16 sources use
            # the 1-cycle/row PE transpose path and a bf16 PSUM tile.
            if src_ap.dtype == BF16:
                pt6 = ptr_pool.tile([P, DT, P], BF16, tag="tr6b")
                tid = ident_bf
            else:
                pt6 = ptr_pool.tile([P, DT, P], F32, tag="tr6")
                tid = ident
            for dt in range(DT):
                nc.tensor.transpose(pt6[:, dt, :],
                                    src_ap[:, dt * P:(dt + 1) * P], tid)
            nc.scalar.activation(out=dst_ap, in_=pt6, func=AF.Copy)

        with ExitStack() as kvs:  # K^T / V live through phases 1-3
            kvp = kvs.enter_context(tc.tile_pool(name="kvp", bufs=1))

            with ExitStack() as kvw:  # weights + LN1 consts for phases 1-2
                wkv = kvw.enter_context(tc.tile_pool(name="wkv", bufs=1))
                wk_sb = wkv.tile([P, DT, D], BF16)
                nc.sync.dma_start(out=wk_sb,
                                  in_=wk_d.rearrange("(t p) f -> p t f", p=P))
                wv_sb = wkv.tile([P, DT, D], BF16)
                nc.sync.dma_start(out=wv_sb,
                                  in_=wv_d.rearrange("(t p) f -> p t f", p=P))
                g1_t = wkv.tile([P, D], F32)
                nc.gpsimd.dma_start(out=g1_t, in_=g1_d.to_broadcast([P, D]))
                be1_t = wkv.tile([P, D], F32)
                nc.gpsimd.dma_start(out=be1_t, in_=be1_d.to_broadcast([P, D]))
                bv_t = wkv.tile([P, D], F32)
                nc.gpsimd.dma_start(out=bv_t, in_=bv_d.to_broadcast([P, D]))
                bv_h = bv_t.rearrange("p (h d) -> p h d", h=H)

                # ---- phase 1: own rows — LN1 + Q/K/V projections ----
                with ExitStack() as p1:
                    wqp = p1.enter_context(tc.tile_pool(name="wqp", bufs=1))
                    wrk = p1.enter_context(tc.tile_pool(name="wrk1", bufs=2))
                    ptr = p1.enter_context(
                        tc.tile_pool(name="ptr1", bufs=2, space="PSUM"))
                    pmm = p1.enter_context(
                        tc.tile_pool(name="pmm1", bufs=2, space="PSUM"))
                    pvv = p1.enter_context(
                        tc.tile_pool(name="pvv1", bufs=1, space="PSUM"))

                    wq_sb = wqp.tile([P, DT, D], BF16)
                    nc.sync.dma_start(
                        out=wq_sb, in_=wq_d.rearrange("(t p) f -> p t f", p=P))
                    for i in range(QB):
                        lnq = wrk.tile([P, D], BF16, tag="lnq")
                        layer_norm(wrk, xq_sb[:, i, :], g1_t, be1_t, lnq)
                        transpose_row(ptr, lnq_T[:, :, i * P:(i + 1) * P], lnq)
                    for ct in range(DT):
                        pq = pmm.tile([P, QB * P], F32, tag="mm512")
                        for dt in range(DT):
                            nc.tensor.matmul(
                                pq, wq_sb[:, dt, ct * P:(ct + 1) * P],
                                lnq_T[:, dt, :], start=(dt == 0),
                                stop=(dt == DT - 1))
                        nc.scalar.activation(out=q_T[:, ct, :], in_=pq,
                                             func=AF.Identity,
                                             bias=bq_t[:, ct:ct + 1], scale=1.0)

                # ---- phase 2: K/V build over the full sequence ----
                k_T = kvp.tile([P, DT, S], BF16)        # 48KB/partition
                v_sb = kvp.tile([P, NB, H, HD1], BF16)  # 48.75KB/partition
                nc.vector.memset(v_sb[:, :, :, HD:HD1], 1.0)
                with ExitStack() as p2:
                    wrk = p2.enter_context(tc.tile_pool(name="wrk2", bufs=2))
                    lnsp = p2.enter_context(tc.tile_pool(name="lnsp", bufs=1))
                    ptr = p2.enter_context(
                        tc.tile_pool(name="ptr2", bufs=2, space="PSUM"))
                    pmm = p2.enter_context(
                        tc.tile_pool(name="pmm2", bufs=2, space="PSUM"))
                    pvv = p2.enter_context(
                        tc.tile_pool(name="pvv2", bufs=1, space="PSUM"))
                    for sb in range(8):
                        lns_T = lnsp.tile([P, DT, 512], BF16, tag="lns_T")
                        xs = wrk.tile([P, 4, D], BF16, tag="xs")
                        nc.sync.dma_start(
                            out=xs,
                            in_=x_d[sb * 512:(sb + 1) * 512, :].rearrange(
                                "(r p) f -> p r f", p=P))
                        for r in range(4):
                            lnb = wrk.tile([P, D], BF16, tag="lnb")
                            layer_norm(wrk, xs[:, r, :], g1_t, be1_t, lnb)
                            transpose_row(ptr, lns_T[:, :, r * P:(r + 1) * P],
                                          lnb)
                        for ct in range(DT):
                            pk = pmm.tile([P, 512], F32, tag="mm512")
                            for dt in range(DT):
                                nc.tensor.matmul(
                                    pk, wk_sb[:, dt, ct * P:(ct + 1) * P],
                                    lns_T[:, dt, :], start=(dt == 0),
                                    stop=(dt == DT - 1))
                            nc.scalar.activation(
                                out=k_T[:, ct, sb * 512:(sb + 1) * 512], in_=pk,
                                func=AF.Identity, bias=bk_t[:, ct:ct + 1],
                                scale=1.0)
                        for r in range(4):
                            blk = sb * 4 + r
                            pv = pvv.tile([P, D], F32, tag="pv")
                            for cols in (slice(0, 512), slice(512, 768)):
                                for dt in range(DT):
                                    nc.tensor.matmul(
                                        pv[:, cols],
                                        lns_T[:, dt, r * P:(r + 1) * P],
                                        wv_sb[:, dt, cols], start=(dt == 0),
                                        stop=(dt == DT - 1))
                            if trivial_affine:
                                nc.vector.tensor_copy(
                                    out=v_sb[:, blk, :, 0:HD],
                                    in_=pv.rearrange("p (h d) -> p h d", h=H))
                            else:
                                nc.vector.tensor_add(
                                    out=v_sb[:, blk, :, 0:HD],
                                    in0=pv.rearrange("p (h d) -> p h d", h=H),
                                    in1=bv_h)

            # ---- phase 3: causal attention ----
            with ExitStack() as p3:
                kmp = p3.enter_context(tc.tile_pool(name="kmp", bufs=1))
                killm_t = kmp.tile([P, NPAIR * P], BF16)
                nc.sync.dma_start(out=killm_t, in_=killm_d)
                att = p3.enter_context(tc.tile_pool(name="att", bufs=3))
                pms = p3.enter_context(
                    tc.tile_pool(name="pms", bufs=2, space="PSUM"))
                pbig = p3.enter_context(
                    tc.tile_pool(name="pbig", bufs=2, space="PSUM"))
                for i in range(QB):
                    vals_a = pbig.tile([P, 6 * HD1], F32, tag="vals_a")
                    vals_b = pbig.tile([P, 6 * HD1], F32, tag="vals_b")
                    nj = 8 * i + 8
                    for h in range(H):
                        vals = vals_a if h < 6 else vals_b
                        hh = h % 6
                        po = (h % 2) * HD
                        ct = h // 2
                        for g in range(nj // 8):
                            t0 = TOFF[i] + 8 * g
                            ps8 = pms.tile([P, 8 * P], F32, tag="mm1024s")
                            for jj in range(8):
                                j = 8 * g + jj
                                nc.tensor.matmul(
                                    ps8[:, jj * P:(jj + 1) * P],
                                    k_T[po:po + HD, ct, j * P:(j + 1) * P],
                                    q_T[po:po + HD, ct, i * P:(i + 1) * P],
                                    start=True, stop=True)
                            pT8 = att.tile([P, 8 * P], BF16, tag="pT8")
                            nc.scalar.activation(out=pT8, in_=ps8, func=AF.Exp,
                                                 scale=SCALE)
                            nc.vector.tensor_mul(
                                out=pT8, in0=pT8,
                                in1=killm_t[:, t0 * P:(t0 + 8) * P])
                            for jj in range(8):
                                j = 8 * g + jj
                                nc.tensor.matmul(
                                    vals[:, hh * HD1:(hh + 1) * HD1],
                                    pT8[:, jj * P:(jj + 1) * P],
                                    v_sb[:, j, h, :],
                                    start=(j == 0), stop=(j == nj - 1))
                        rs = att.tile([P, 1], F32, tag="rs")
                        nc.vector.reciprocal(
                            out=rs, in_=vals[:, hh * HD1 + HD:hh * HD1 + HD1])
                        nc.vector.tensor_scalar_mul(
                            out=attn_sb[:, i, h * HD:(h + 1) * HD],
                            in0=vals[:, hh * HD1:hh * HD1 + HD], scalar1=rs)

        # ---- phases 4+5: Wo + residual + LN2 + FFN + residual ----
        with ExitStack() as p45:
            big = p45.enter_context(tc.tile_pool(name="big45", bufs=1))
            wrk = p45.enter_context(tc.tile_pool(name="wrk45", bufs=2))
            w1s = p45.enter_context(tc.tile_pool(name="w1s", bufs=2))
            w2s = p45.enter_context(tc.tile_pool(name="w2s", bufs=2))
            ptr = p45.enter_context(
                tc.tile_pool(name="ptr45", bufs=2, space="PSUM"))
            pmm = p45.enter_context(
                tc.tile_pool(name="pmm45", bufs=2, space="PSUM"))

            g2_t = big.tile([P, D], F32)
            nc.gpsimd.dma_start(out=g2_t, in_=g2_d.to_broadcast([P, D]))
            be2_t = big.tile([P, D], F32)
            nc.gpsimd.dma_start(out=be2_t, in_=be2_d.to_broadcast([P, D]))
            x1_sb = big.tile([P, QB, D], F32)
            out_sb = big.tile([P, QB, D], F32)
            wo_sb = big.tile([P, DT, D], BF16)
            nc.sync.dma_start(out=wo_sb,
                              in_=wo_d.rearrange("(t p) f -> p t f", p=P))
            attn_T = big.tile([P, DT, QB * P], BF16)
            for i in range(QB):
                transpose_row(ptr, attn_T[:, :, i * P:(i + 1) * P],
                              attn_sb[:, i, :])
            wo_T = big.tile([P, DT, QB * P], F32, tag="big512")
            for ot in range(DT):
                pm = pmm.tile([P, QB * P], F32, tag="mm512")
                for dt in range(DT):
                    nc.tensor.matmul(
                        pm, wo_sb[:, dt, ot * P:(ot + 1) * P],
                        attn_T[:, dt, :], start=(dt == 0), stop=(dt == DT - 1))
                nc.scalar.activation(out=wo_T[:, ot, :], in_=pm,
                                     func=AF.Identity,
                                     bias=bo_t[:, ot:ot + 1], scale=1.0)
            ln2_T = big.tile([P, DT, QB * P], BF16)
            for i in range(QB):
                for ot in range(DT):
                    pt = ptr.tile([P, P], F32, tag="tr")
                    nc.tensor.transpose(pt, wo_T[:, ot, i * P:(i + 1) * P],
                                        ident)
                    nc.vector.tensor_add(
                        out=x1_sb[:, i, ot * P:(ot + 1) * P],
                        in0=pt, in1=xq_sb[:, i, ot * P:(ot + 1) * P])
                ln2 = wrk.tile([P, D], BF16, tag="ln2")
                layer_norm(wrk, x1_sb[:, i, :], g2_t, be2_t, ln2)
                transpose_row(ptr, ln2_T[:, :, i * P:(i + 1) * P], ln2)

            h_T = big.tile([P, FT, QB * P], BF16)
            for ft in range(FT):
                if ft % 6 == 0:
                    w1t = w1s.tile([P, DT, 6 * P], BF16, tag="w1t")
                    nc.sync.dma_start(
                        out=w1t,
                        in_=w1_d.rearrange("(t p) f -> p t f", p=P)[
                            :, :, ft * P:(ft + 6) * P])
                fl = ft % 6
                pm = pmm.tile([P, QB * P], F32, tag="mm512")
                for dt in range(DT):
                    nc.tensor.matmul(pm, w1t[:, dt, fl * P:(fl + 1) * P],
                                     ln2_T[:, dt, :],
                                     start=(dt == 0), stop=(dt == DT - 1))
                if GELU_SIGMOID_APPROX:
                    t1 = wrk.tile([P, QB * P], F32, tag="gel1")
                    nc.scalar.activation(out=t1, in_=pm, func=AF.Identity,
                                         bias=b1_t[:, ft:ft + 1], scale=1.0)
                    t2 = wrk.tile([P, QB * P], F32, tag="gel2")
                    nc.scalar.activation(out=t2, in_=t1, func=AF.Sigmoid,
                                         scale=1.702)
                    nc.vector.tensor_mul(out=h_T[:, ft, :], in0=t1, in1=t2)
                else:
                    nc.scalar.activation(out=h_T[:, ft, :], in_=pm,
                                         func=AF.Gelu,
                                         bias=b1_t[:, ft:ft + 1], scale=1.0)
            f_T = big.tile([P, DT, QB * P], F32, tag="big512")
            for ot in range(DT):
                w2t = w2s.tile([P, FT, P], BF16, tag="w2t")
                nc.sync.dma_start(
                    out=w2t,
                    in_=w2_d.rearrange("(t p) f -> p t f", p=P)[
                        :, :, ot * P:(ot + 1) * P])
                pm = pmm.tile([P, QB * P], F32, tag="mm512")
                for ft in range(FT):
                    nc.tensor.matmul(pm, w2t[:, ft, :], h_T[:, ft, :],
                                     start=(ft == 0), stop=(ft == FT - 1))
                nc.scalar.activation(out=f_T[:, ot, :], in_=pm,
                                     func=AF.Identity,
                                     bias=b2_t[:, ot:ot + 1], scale=1.0)
            for i in range(QB):
                for ot in range(DT):
                    pt = ptr.tile([P, P], F32, tag="tr")
                    nc.tensor.transpose(pt, f_T[:, ot, i * P:(i + 1) * P],
                                        ident)
                    nc.vector.tensor_add(
                        out=out_sb[:, i, ot * P:(ot + 1) * P],
                        in0=pt, in1=x1_sb[:, i, ot * P:(ot + 1) * P])
            nc.sync.dma_start(
                out=out_d.rearrange("(r p) f -> p r f", p=P), in_=out_sb)

    nc.compile()
    return nc


def _prep_inputs(x, gamma1, beta1, Wqkv, bqkv, Wo, bo, gamma2, beta2,
                 W1, b1, W2, b2):
    """Build the 8 per-core input maps (all numpy, host-side)."""
    bf = ml_dtypes.bfloat16
    x2 = np.ascontiguousarray(np.asarray(x).reshape(S, D), dtype=np.float32)
    W4 = np.asarray(Wqkv).reshape(D, H, 3, HD)
    wq = np.ascontiguousarray(W4[:, :, 0, :].reshape(D, D)).astype(bf)
    wk = np.ascontiguousarray(W4[:, :, 1, :].reshape(D, D)).astype(bf)
    wv = np.ascontiguousarray(W4[:, :, 2, :].reshape(D, D)).astype(bf)
    b4 = np.asarray(bqkv).reshape(H, 3, HD)

    common = {
        "x_full": x2.astype(bf),
        "wq": wq, "wk": wk, "wv": wv,
        "wo": np.asarray(Wo, dtype=np.float32).astype(bf),
        "w1": np.asarray(W1, dtype=np.float32).astype(bf),
        "w2": np.asarray(W2, dtype=np.float32).astype(bf),
        "g1": np.asarray(gamma1, np.float32).reshape(1, D).copy(),
        "be1": np.asarray(beta1, np.float32).reshape(1, D).copy(),
        "g2": np.asarray(gamma2, np.float32).reshape(1, D).copy(),
        "be2": np.asarray(beta2, np.float32).reshape(1, D).copy(),
        "bq_t": np.ascontiguousarray(
            b4[:, 0, :].reshape(DT, P).T).astype(np.float32),
        "bk_t": np.ascontiguousarray(
            b4[:, 1, :].reshape(DT, P).T).astype(np.float32),
        "bv_r": np.ascontiguousarray(
            b4[:, 2, :].reshape(1, D)).astype(np.float32),
        "bo_t": np.ascontiguousarray(
            np.asarray(bo).reshape(DT, P).T).astype(np.float32),
        "b1_t": np.ascontiguousarray(
            np.asarray(b1).reshape(FT, P).T).astype(np.float32),
        "b2_t": np.ascontiguousarray(
            np.asarray(b2).reshape(DT, P).T).astype(np.float32),
        # tri[p, f] = 1 where k_local (p) <= q_local (f)
        "tri": np.ascontiguousarray(
            np.tril(np.ones((P, P), np.float32)).T).astype(bf),
    }

    xb = x2.reshape(NB, P, D)
    in_maps = []
    for c in range(NCORES):
        blocks = [c + 8 * i for i in range(QB)]
        xq = np.ascontiguousarray(xb[blocks].reshape(QB * P, D))
        selb = np.zeros((P, NPAIR), np.float32)
        for i in range(QB):
            for j in range(8 * i + 8):
                if j >= c + 8 * i:
                    selb[:, TOFF[i] + j] = NEG
        killm = np.zeros((P, NPAIR * P), np.float32)
        triangle = np.tril(np.ones((P, P), np.float32)).T  # 1 where k<=q
        for i in range(QB):
            for j in range(8 * i + 8):
                t = TOFF[i] + j
                if j < c + 8 * i:
                    killm[:, t * P:(t + 1) * P] = 1.0
                elif j == c + 8 * i:
                    killm[:, t * P:(t + 1) * P] = triangle
        m = dict(common)
        m["xq"] = xq
        m["selbias"] = selb
        m["killm"] = killm.astype(bf)
        in_maps.append(m)
    return in_maps


def kernel(**inputs):
    trivial = bool(
        np.all(np.asarray(inputs["gamma1"]) == 1.0)
        and np.all(np.asarray(inputs["beta1"]) == 0.0)
        and np.all(np.asarray(inputs["gamma2"]) == 1.0)
        and np.all(np.asarray(inputs["beta2"]) == 0.0)
        and not np.asarray(inputs["bqkv"]).reshape(H, 3, HD)[:, 2, :].any())
    key = ("nc", trivial)
    nc = _CACHE.get(key)
    if nc is None:
        nc = _build(trivial_affine=trivial)
        _CACHE[key] = nc
    in_maps = _prep_inputs(**inputs)
    res = run_bass_kernel_spmd(nc, in_maps, list(range(NCORES)))
    out = np.zeros((S, D), np.float32)
    ob = out.reshape(NB, P, D)
    for c in range(NCORES):
        o = np.asarray(res.results[c]["out"]).reshape(QB, P, D)
        for i in range(QB):
            ob[c + 8 * i] = o[i]
    return out.reshape(1, S, D)


# revision 20
# speedup vs baseline: 1.0422x; 1.0422x over previous
"""Trainium2 Bass kernel for a GPT-style decoder block (d=768, H=12, S=4096, FFN=3072).

Sharding: 8-way SPMD over query-row blocks, cyclic assignment — core c owns the
four 128-row blocks {c, c+8, c+16, c+24}.  Every core computes LN1 + K/V
projections for the full sequence locally (no collectives), then causal
attention + output projection + LN2 + FFN for its own 512 query rows.

All per-core variation is carried by input *data* (own-rows tensor, mask-bias
table); the program itself is identical on all 8 cores and fully static.

Matmuls run with bf16 operands and fp32 PSUM accumulation (measured pipeline
error vs the fp32 reference: ~1e-3 absmax-relative).  LayerNorm, softmax,
GELU and residuals are fp32.
"""

import os
import sys
from contextlib import ExitStack

import numpy as np
import ml_dtypes

for _p in ("/opt/trn_rl_repo", "/opt/pypackages"):
    if os.path.isdir(_p) and _p not in sys.path:
        sys.path.append(_p)

import concourse.bacc as bacc
import concourse.tile as tile
from concourse import mybir
from concourse.bass_utils import run_bass_kernel_spmd
from concourse.masks import make_identity

P = 128
D = 768
DT = D // P            # 6 d-tiles
H = 12
HD = 64
S = 4096
NB = S // P            # 32 sequence blocks
HID = 3072
FT = HID // P          # 24 hidden f-tiles
QB = 4                 # q-blocks per core
NCORES = 8
EPS = 1e-5
NEG = -1.0e9
SCALE = 0.125          # 1/sqrt(HD)
TOFF = [0, 8, 24, 48]  # flat (i, j) pair index offsets; 80 static pairs total
NPAIR = 80
HD1 = HD + 1           # V columns + appended ones column (softmax denominator)

F32 = mybir.dt.float32
BF16 = mybir.dt.bfloat16
AF = mybir.ActivationFunctionType

_CACHE = {}

# CoreSim does not implement the Gelu activation table; this flag substitutes
# x*sigmoid(1.702x) so the simulator can validate kernel structure.  Hardware
# runs always use the real (erf) Gelu.
GELU_SIGMOID_APPROX = False


def _build(trivial_affine=False):
    nc = bacc.Bacc("TRN2", target_bir_lowering=False, debug=False,
                   num_devices=NCORES)

    # ---- DRAM I/O ----
    x_d = nc.dram_tensor("x_full", [S, D], BF16, kind="ExternalInput").ap()
    xq_d = nc.dram_tensor("xq", [QB * P, D], F32, kind="ExternalInput").ap()
    wq_d = nc.dram_tensor("wq", [D, D], BF16, kind="ExternalInput").ap()
    wk_d = nc.dram_tensor("wk", [D, D], BF16, kind="ExternalInput").ap()
    wv_d = nc.dram_tensor("wv", [D, D], BF16, kind="ExternalInput").ap()
    wo_d = nc.dram_tensor("wo", [D, D], BF16, kind="ExternalInput").ap()
    w1_d = nc.dram_tensor("w1", [D, HID], BF16, kind="ExternalInput").ap()
    w2_d = nc.dram_tensor("w2", [HID, D], BF16, kind="ExternalInput").ap()
    g1_d = nc.dram_tensor("g1", [1, D], F32, kind="ExternalInput").ap()
    be1_d = nc.dram_tensor("be1", [1, D], F32, kind="ExternalInput").ap()
    g2_d = nc.dram_tensor("g2", [1, D], F32, kind="ExternalInput").ap()
    be2_d = nc.dram_tensor("be2", [1, D], F32, kind="ExternalInput").ap()
    bq_d = nc.dram_tensor("bq_t", [P, DT], F32, kind="ExternalInput").ap()
    bk_d = nc.dram_tensor("bk_t", [P, DT], F32, kind="ExternalInput").ap()
    bv_d = nc.dram_tensor("bv_r", [1, D], F32, kind="ExternalInput").ap()
    bo_d = nc.dram_tensor("bo_t", [P, DT], F32, kind="ExternalInput").ap()
    b1_d = nc.dram_tensor("b1_t", [P, FT], F32, kind="ExternalInput").ap()
    b2_d = nc.dram_tensor("b2_t", [P, DT], F32, kind="ExternalInput").ap()
    selb_d = nc.dram_tensor("selbias", [P, NPAIR], F32, kind="ExternalInput").ap()
    killm_d = nc.dram_tensor("killm", [P, NPAIR * P], BF16,
                             kind="ExternalInput").ap()
    tri_d = nc.dram_tensor("tri", [P, P], BF16, kind="ExternalInput").ap()
    out_d = nc.dram_tensor("out", [QB * P, D], F32, kind="ExternalOutput").ap()

    with tile.TileContext(nc) as tc, ExitStack() as root:
        singles = root.enter_context(tc.tile_pool(name="singles", bufs=1))

        # ---- constants + activations resident for the whole kernel ----
        ident = singles.tile([P, P], F32)
        make_identity(nc, ident)
        ident_bf = singles.tile([P, P], BF16)
        make_identity(nc, ident_bf)
        eps_t = singles.tile([P, 1], F32)
        nc.vector.memset(eps_t, EPS)
        bq_t = singles.tile([P, DT], F32)
        nc.sync.dma_start(out=bq_t, in_=bq_d)
        bk_t = singles.tile([P, DT], F32)
        nc.sync.dma_start(out=bk_t, in_=bk_d)
        bo_t = singles.tile([P, DT], F32)
        nc.sync.dma_start(out=bo_t, in_=bo_d)
        b1_t = singles.tile([P, FT], F32)
        nc.sync.dma_start(out=b1_t, in_=b1_d)
        b2_t = singles.tile([P, DT], F32)
        nc.sync.dma_start(out=b2_t, in_=b2_d)
        tri_t = singles.tile([P, P], BF16)
        nc.sync.dma_start(out=tri_t, in_=tri_d)

        xq_sb = singles.tile([P, QB, D], F32)        # raw own rows (residual)
        nc.sync.dma_start(out=xq_sb, in_=xq_d.rearrange("(r p) f -> p r f", p=P))
        lnq_T = singles.tile([P, DT, QB * P], BF16)  # LN1(own rows)^T
        q_T = singles.tile([P, DT, QB * P], BF16)    # Q^T (feature-major)
        attn_sb = singles.tile([P, QB, D], BF16)     # attention output

        def layer_norm(pool, x_ap, g_tile, b_tile, out_ap):
            stats = pool.tile([P, 3, nc.vector.BN_STATS_DIM], F32, tag="lns")
            mv = pool.tile([P, nc.vector.BN_AGGR_DIM], F32, tag="lnm")
            xg = x_ap.rearrange("p (g f) -> p g f", g=3)
            for g in range(3):
                nc.vector.bn_stats(out=stats[:, g, :], in_=xg[:, g, :])
            nc.vector.bn_aggr(out=mv, in_=stats)
            rstd = pool.tile([P, 1], F32, tag="lnr")
            nc.scalar.activation(out=rstd, in_=mv[:, 1:2], func=AF.Sqrt,
                                 bias=eps_t, scale=1.0)
            nc.vector.reciprocal(out=rstd, in_=rstd)
            nc.vector.tensor_scalar(out=out_ap, in0=x_ap, scalar1=mv[:, 0:1],
                                    scalar2=rstd, op0=mybir.AluOpType.subtract,
                                    op1=mybir.AluOpType.mult)
            if not trivial_affine:
                nc.vector.tensor_mul(out=out_ap, in0=out_ap, in1=g_tile)
                nc.vector.tensor_add(out=out_ap, in0=out_ap, in1=b_tile)

        def transpose_block(ptr_pool, dst_ap, src_ap):
            pt = ptr_pool.tile([P, P], F32, tag="tr")
            nc.tensor.transpose(pt, src_ap, ident)
            nc.scalar.activation(out=dst_ap, in_=pt, func=AF.Copy)

        def transpose_row(ptr_pool, dst_ap, src_ap):
            # Transpose a [128, 768] row-major block into dst [128, 6, 128]
            # (feature-major) with one batched PSUM drain.  bf16 sources use
            # the 1-cycle/row PE transpose path and a bf16 PSUM tile.
            if src_ap.dtype == BF16:
                pt6 = ptr_pool.tile([P, DT, P], BF16, tag="tr6b")
                tid = ident_bf
            else:
                pt6 = ptr_pool.tile([P, DT, P], F32, tag="tr6")
                tid = ident
            for dt in range(DT):
                nc.tensor.transpose(pt6[:, dt, :],
                                    src_ap[:, dt * P:(dt + 1) * P], tid)
            nc.scalar.activation(out=dst_ap, in_=pt6, func=AF.Copy)

        with ExitStack() as kvs:  # K^T / V live through phases 1-3
            kvp = kvs.enter_context(tc.tile_pool(name="kvp", bufs=1))

            with ExitStack() as kvw:  # weights + LN1 consts for phases 1-2
                wkv = kvw.enter_context(tc.tile_pool(name="wkv", bufs=1))
                wk_sb = wkv.tile([P, DT, D], BF16)
                nc.sync.dma_start(out=wk_sb,
                                  in_=wk_d.rearrange("(t p) f -> p t f", p=P))
                wv_sb = wkv.tile([P, DT, D], BF16)
                nc.sync.dma_start(out=wv_sb,
                                  in_=wv_d.rearrange("(t p) f -> p t f", p=P))
                g1_t = wkv.tile([P, D], F32)
                nc.gpsimd.dma_start(out=g1_t, in_=g1_d.to_broadcast([P, D]))
                be1_t = wkv.tile([P, D], F32)
                nc.gpsimd.dma_start(out=be1_t, in_=be1_d.to_broadcast([P, D]))
                bv_t = wkv.tile([P, D], F32)
                nc.gpsimd.dma_start(out=bv_t, in_=bv_d.to_broadcast([P, D]))
                bv_h = bv_t.rearrange("p (h d) -> p h d", h=H)

                # ---- phase 1: own rows — LN1 + Q/K/V projections ----
                with ExitStack() as p1:
                    wqp = p1.enter_context(tc.tile_pool(name="wqp", bufs=1))
                    wrk = p1.enter_context(tc.tile_pool(name="wrk1", bufs=2))
                    ptr = p1.enter_context(
                        tc.tile_pool(name="ptr1", bufs=2, space="PSUM"))
                    pmm = p1.enter_context(
                        tc.tile_pool(name="pmm1", bufs=2, space="PSUM"))
                    pvv = p1.enter_context(
                        tc.tile_pool(name="pvv1", bufs=1, space="PSUM"))

                    wq_sb = wqp.tile([P, DT, D], BF16)
                    nc.sync.dma_start(
                        out=wq_sb, in_=wq_d.rearrange("(t p) f -> p t f", p=P))
                    for i in range(QB):
                        lnq = wrk.tile([P, D], BF16, tag="lnq")
                        layer_norm(wrk, xq_sb[:, i, :], g1_t, be1_t, lnq)
                        transpose_row(ptr, lnq_T[:, :, i * P:(i + 1) * P], lnq)
                    for ct in range(DT):
                        pq = pmm.tile([P, QB * P], F32, tag="mm512")
                        for dt in range(DT):
                            nc.tensor.matmul(
                                pq, wq_sb[:, dt, ct * P:(ct + 1) * P],
                                lnq_T[:, dt, :], start=(dt == 0),
                                stop=(dt == DT - 1))
                        nc.scalar.activation(out=q_T[:, ct, :], in_=pq,
                                             func=AF.Identity,
                                             bias=bq_t[:, ct:ct + 1], scale=1.0)

                # ---- phase 2: K/V build over the full sequence ----
                k_T = kvp.tile([P, DT, S], BF16)        # 48KB/partition
                v_sb = kvp.tile([P, NB, H, HD1], BF16)  # 48.75KB/partition
                nc.vector.memset(v_sb[:, :, :, HD:HD1], 1.0)
                with ExitStack() as p2:
                    wrk = p2.enter_context(tc.tile_pool(name="wrk2", bufs=2))
                    lnsp = p2.enter_context(tc.tile_pool(name="lnsp", bufs=1))
                    ptr = p2.enter_context(
                        tc.tile_pool(name="ptr2", bufs=2, space="PSUM"))
                    pmm = p2.enter_context(
                        tc.tile_pool(name="pmm2", bufs=2, space="PSUM"))
                    pvv = p2.enter_context(
                        tc.tile_pool(name="pvv2", bufs=2, space="PSUM"))
                    for sb in range(8):
                        lns_T = lnsp.tile([P, DT, 512], BF16, tag="lns_T")
                        xs = wrk.tile([P, 4, D], BF16, tag="xs")
                        nc.sync.dma_start(
                            out=xs,
                            in_=x_d[sb * 512:(sb + 1) * 512, :].rearrange(
                                "(r p) f -> p r f", p=P))
                        for r in range(4):
                            lnb = wrk.tile([P, D], BF16, tag="lnb")
                            layer_norm(wrk, xs[:, r, :], g1_t, be1_t, lnb)
                            transpose_row(ptr, lns_T[:, :, r * P:(r + 1) * P],
                                          lnb)
                        for ct in range(DT):
                            pk = pmm.tile([P, 512], F32, tag="mm512")
                            for dt in range(DT):
                                nc.tensor.matmul(
                                    pk, wk_sb[:, dt, ct * P:(ct + 1) * P],
                                    lns_T[:, dt, :], start=(dt == 0),
                                    stop=(dt == DT - 1))
                            nc.scalar.activation(
                                out=k_T[:, ct, sb * 512:(sb + 1) * 512], in_=pk,
                                func=AF.Identity, bias=bk_t[:, ct:ct + 1],
                                scale=1.0)
                        for r in range(4):
                            blk = sb * 4 + r
                            pv = pvv.tile([P, D], F32, tag="pv")
                            for cols in (slice(0, 512), slice(512, 768)):
                                for dt in range(DT):
                                    nc.tensor.matmul(
                                        pv[:, cols],
                                        lns_T[:, dt, r * P:(r + 1) * P],
                                        wv_sb[:, dt, cols], start=(dt == 0),
                                        stop=(dt == DT - 1))
                            if trivial_affine:
                                nc.vector.tensor_copy(
                                    out=v_sb[:, blk, :, 0:HD],
                                    in_=pv.rearrange("p (h d) -> p h d", h=H))
                            else:
                                nc.vector.tensor_add(
                                    out=v_sb[:, blk, :, 0:HD],
                                    in0=pv.rearrange("p (h d) -> p h d", h=H),
                                    in1=bv_h)

            # ---- phase 3: causal attention ----
            with ExitStack() as p3:
                kmp = p3.enter_context(tc.tile_pool(name="kmp", bufs=1))
                killm_t = kmp.tile([P, NPAIR * P], BF16)
                nc.sync.dma_start(out=killm_t, in_=killm_d)
                att = p3.enter_context(tc.tile_pool(name="att", bufs=3))
                pms = p3.enter_context(
                    tc.tile_pool(name="pms", bufs=2, space="PSUM"))
                pbig = p3.enter_context(
                    tc.tile_pool(name="pbig", bufs=2, space="PSUM"))
                for i in range(QB):
                    vals_a = pbig.tile([P, 6 * HD1], F32, tag="vals_a")
                    vals_b = pbig.tile([P, 6 * HD1], F32, tag="vals_b")
                    nj = 8 * i + 8
                    for h in range(H):
                        vals = vals_a if h < 6 else vals_b
                        hh = h % 6
                        po = (h % 2) * HD
                        ct = h // 2
                        for g in range(nj // 8):
                            t0 = TOFF[i] + 8 * g
                            ps8 = pms.tile([P, 8 * P], F32, tag="mm1024s")
                            for jj in range(8):
                                j = 8 * g + jj
                                nc.tensor.matmul(
                                    ps8[:, jj * P:(jj + 1) * P],
                                    k_T[po:po + HD, ct, j * P:(j + 1) * P],
                                    q_T[po:po + HD, ct, i * P:(i + 1) * P],
                                    start=True, stop=True)
                            pT8 = att.tile([P, 8 * P], BF16, tag="pT8")
                            nc.scalar.activation(out=pT8, in_=ps8, func=AF.Exp,
                                                 scale=SCALE)
                            nc.vector.tensor_mul(
                                out=pT8, in0=pT8,
                                in1=killm_t[:, t0 * P:(t0 + 8) * P])
                            for jj in range(8):
                                j = 8 * g + jj
                                nc.tensor.matmul(
                                    vals[:, hh * HD1:(hh + 1) * HD1],
                                    pT8[:, jj * P:(jj + 1) * P],
                                    v_sb[:, j, h, :],
                                    start=(j == 0), stop=(j == nj - 1))
                        rs = att.tile([P, 1], F32, tag="rs")
                        nc.vector.reciprocal(
                            out=rs, in_=vals[:, hh * HD1 + HD:hh * HD1 + HD1])
                        nc.vector.tensor_scalar_mul(
                            out=attn_sb[:, i, h * HD:(h + 1) * HD],
                            in0=vals[:, hh * HD1:hh * HD1 + HD], scalar1=rs)

        # ---- phases 4+5: Wo + residual + LN2 + FFN + residual ----
        with ExitStack() as p45:
            big = p45.enter_context(tc.tile_pool(name="big45", bufs=1))
            wrk = p45.enter_context(tc.tile_pool(name="wrk45", bufs=2))
            w1s = p45.enter_context(tc.tile_pool(name="w1s", bufs=2))
            w2s = p45.enter_context(tc.tile_pool(name="w2s", bufs=2))
            ptr = p45.enter_context(
                tc.tile_pool(name="ptr45", bufs=2, space="PSUM"))
            pmm = p45.enter_context(
                tc.tile_pool(name="pmm45", bufs=2, space="PSUM"))

            g2_t = big.tile([P, D], F32)
            nc.gpsimd.dma_start(out=g2_t, in_=g2_d.to_broadcast([P, D]))
            be2_t = big.tile([P, D], F32)
            nc.gpsimd.dma_start(out=be2_t, in_=be2_d.to_broadcast([P, D]))
            x1_sb = big.tile([P, QB, D], F32)
            out_sb = big.tile([P, QB, D], F32)
            wo_sb = big.tile([P, DT, D], BF16)
            nc.sync.dma_start(out=wo_sb,
                              in_=wo_d.rearrange("(t p) f -> p t f", p=P))
            attn_T = big.tile([P, DT, QB * P], BF16)
            for i in range(QB):
                transpose_row(ptr, attn_T[:, :, i * P:(i + 1) * P],
                              attn_sb[:, i, :])
            wo_T = big.tile([P, DT, QB * P], F32, tag="big512")
            for ot in range(DT):
                pm = pmm.tile([P, QB * P], F32, tag="mm512")
                for dt in range(DT):
                    nc.tensor.matmul(
                        pm, wo_sb[:, dt, ot * P:(ot + 1) * P],
                        attn_T[:, dt, :], start=(dt == 0), stop=(dt == DT - 1))
                nc.scalar.activation(out=wo_T[:, ot, :], in_=pm,
                                     func=AF.Identity,
                                     bias=bo_t[:, ot:ot + 1], scale=1.0)
            ln2_T = big.tile([P, DT, QB * P], BF16)
            for i in range(QB):
                for ot in range(DT):
                    pt = ptr.tile([P, P], F32, tag="tr")
                    nc.tensor.transpose(pt, wo_T[:, ot, i * P:(i + 1) * P],
                                        ident)
                    nc.vector.tensor_add(
                        out=x1_sb[:, i, ot * P:(ot + 1) * P],
                        in0=pt, in1=xq_sb[:, i, ot * P:(ot + 1) * P])
                ln2 = wrk.tile([P, D], BF16, tag="ln2")
                layer_norm(wrk, x1_sb[:, i, :], g2_t, be2_t, ln2)
                transpose_row(ptr, ln2_T[:, :, i * P:(i + 1) * P], ln2)

            h_T = big.tile([P, FT, QB * P], BF16)
            for ft in range(FT):
                if ft % 6 == 0:
                    w1t = w1s.tile([P, DT, 6 * P], BF16, tag="w1t")
                    nc.sync.dma_start(
                        out=w1t,
                        in_=w1_d.rearrange("(t p) f -> p t f", p=P)[
                            :, :, ft * P:(ft + 6) * P])
                fl = ft % 6
                pm = pmm.tile([P, QB * P], F32, tag="mm512")
                for dt in range(DT):
                    nc.tensor.matmul(pm, w1t[:, dt, fl * P:(fl + 1) * P],
                                     ln2_T[:, dt, :],
                                     start=(dt == 0), stop=(dt == DT - 1))
                if GELU_SIGMOID_APPROX:
                    t1 = wrk.tile([P, QB * P], F32, tag="gel1")
                    nc.scalar.activation(out=t1, in_=pm, func=AF.Identity,
                                         bias=b1_t[:, ft:ft + 1], scale=1.0)
                    t2 = wrk.tile([P, QB * P], F32, tag="gel2")
                    nc.scalar.activation(out=t2, in_=t1, func=AF.Sigmoid,
                                         scale=1.702)
                    nc.vector.tensor_mul(out=h_T[:, ft, :], in0=t1, in1=t2)
                else:
                    nc.scalar.activation(out=h_T[:, ft, :], in_=pm,
                                         func=AF.Gelu,
                                         bias=b1_t[:, ft:ft + 1], scale=1.0)
            f_T = big.tile([P, DT, QB * P], F32, tag="big512")
            for ot in range(DT):
                w2t = w2s.tile([P, FT, P], BF16, tag="w2t")
                nc.sync.dma_start(
                    out=w2t,
                    in_=w2_d.rearrange("(t p) f -> p t f", p=P)[
                        :, :, ot * P:(ot + 1) * P])
                pm = pmm.tile([P, QB * P], F32, tag="mm512")
                for ft in range(FT):
                    nc.tensor.matmul(pm, w2t[:, ft, :], h_T[:, ft, :],
                                     start=(ft == 0), stop=(ft == FT - 1))
                nc.scalar.activation(out=f_T[:, ot, :], in_=pm,
                                     func=AF.Identity,
                                     bias=b2_t[:, ot:ot + 1], scale=1.0)
            for i in range(QB):
                for ot in range(DT):
                    pt = ptr.tile([P, P], F32, tag="tr")
                    nc.tensor.transpose(pt, f_T[:, ot, i * P:(i + 1) * P],
                                        ident)
                    nc.vector.tensor_add(
                        out=out_sb[:, i, ot * P:(ot + 1) * P],
                        in0=pt, in1=x1_sb[:, i, ot * P:(ot + 1) * P])
            nc.sync.dma_start(
                out=out_d.rearrange("(r p) f -> p r f", p=P), in_=out_sb)

    nc.compile()
    return nc


def _prep_inputs(x, gamma1, beta1, Wqkv, bqkv, Wo, bo, gamma2, beta2,
                 W1, b1, W2, b2):
    """Build the 8 per-core input maps (all numpy, host-side)."""
    bf = ml_dtypes.bfloat16
    x2 = np.ascontiguousarray(np.asarray(x).reshape(S, D), dtype=np.float32)
    W4 = np.asarray(Wqkv).reshape(D, H, 3, HD)
    wq = np.ascontiguousarray(W4[:, :, 0, :].reshape(D, D)).astype(bf)
    wk = np.ascontiguousarray(W4[:, :, 1, :].reshape(D, D)).astype(bf)
    wv = np.ascontiguousarray(W4[:, :, 2, :].reshape(D, D)).astype(bf)
    b4 = np.asarray(bqkv).reshape(H, 3, HD)

    common = {
        "x_full": x2.astype(bf),
        "wq": wq, "wk": wk, "wv": wv,
        "wo": np.asarray(Wo, dtype=np.float32).astype(bf),
        "w1": np.asarray(W1, dtype=np.float32).astype(bf),
        "w2": np.asarray(W2, dtype=np.float32).astype(bf),
        "g1": np.asarray(gamma1, np.float32).reshape(1, D).copy(),
        "be1": np.asarray(beta1, np.float32).reshape(1, D).copy(),
        "g2": np.asarray(gamma2, np.float32).reshape(1, D).copy(),
        "be2": np.asarray(beta2, np.float32).reshape(1, D).copy(),
        "bq_t": np.ascontiguousarray(
            b4[:, 0, :].reshape(DT, P).T).astype(np.float32),
        "bk_t": np.ascontiguousarray(
            b4[:, 1, :].reshape(DT, P).T).astype(np.float32),
        "bv_r": np.ascontiguousarray(
            b4[:, 2, :].reshape(1, D)).astype(np.float32),
        "bo_t": np.ascontiguousarray(
            np.asarray(bo).reshape(DT, P).T).astype(np.float32),
        "b1_t": np.ascontiguousarray(
            np.asarray(b1).reshape(FT, P).T).astype(np.float32),
        "b2_t": np.ascontiguousarray(
            np.asarray(b2).reshape(DT, P).T).astype(np.float32),
        # tri[p, f] = 1 where k_local (p) <= q_local (f)
        "tri": np.ascontiguousarray(
            np.tril(np.ones((P, P), np.float32)).T).astype(bf),
    }

    xb = x2.reshape(NB, P, D)
    in_maps = []
    for c in range(NCORES):
        blocks = [c + 8 * i for i in range(QB)]
        xq = np.ascontiguousarray(xb[blocks].reshape(QB * P, D))
        selb = np.zeros((P, NPAIR), np.float32)
        for i in range(QB):
            for j in range(8 * i + 8):
                if j >= c + 8 * i:
                    selb[:, TOFF[i] + j] = NEG
        killm = np.zeros((P, NPAIR * P), np.float32)
        triangle = np.tril(np.ones((P, P), np.float32)).T  # 1 where k<=q
        for i in range(QB):
            for j in range(8 * i + 8):
                t = TOFF[i] + j
                if j < c + 8 * i:
                    killm[:, t * P:(t + 1) * P] = 1.0
                elif j == c + 8 * i:
                    killm[:, t * P:(t + 1) * P] = triangle
        m = dict(common)
        m["xq"] = xq
        m["selbias"] = selb
        m["killm"] = killm.astype(bf)
        in_maps.append(m)
    return in_maps


def kernel(**inputs):
    trivial = bool(
        np.all(np.asarray(inputs["gamma1"]) == 1.0)
        and np.all(np.asarray(inputs["beta1"]) == 0.0)
        and np.all(np.asarray(inputs["gamma2"]) == 1.0)
        and np.all(np.asarray(inputs["beta2"]) == 0.0)
        and not np.asarray(inputs["bqkv"]).reshape(H, 3, HD)[:, 2, :].any())
    key = ("nc", trivial)
    nc = _CACHE.get(key)
    if nc is None:
        nc = _build(trivial_affine=trivial)
        _CACHE[key] = nc
    in_maps = _prep_inputs(**inputs)
    res = run_bass_kernel_spmd(nc, in_maps, list(range(NCORES)))
    out = np.zeros((S, D), np.float32)
    ob = out.reshape(NB, P, D)
    for c in range(NCORES):
        o = np.asarray(res.results[c]["out"]).reshape(QB, P, D)
        for i in range(QB):
            ob[c + 8 * i] = o[i]
    return out.reshape(1, S, D)
